# revision 13
# baseline (speedup 1.0000x reference)
"""CascadedAttentionCell Trainium2 kernel.

Full shapes: inputs [64, 512, 1024] f32, prev_state [64, 1024] f32,
Wa [1024,1024], Ua [1024,1024], Va [1024,1], Ba [1,1024].
Output: context vector [64, 1024] f32.

Sharding: data-parallel over batch across 8 NeuronCores (8 batches/core);
weights replicated.

Per-core plan (B=8 local batches, T=512, D=1024, OUT=1024, P=128):
 - prep: Ua -> SBUF fp16; Wa -> SBUF f32; prev_state transposed via PE;
   WaS^T = Wa^T @ prev^T (fp32 matmuls, N=8); + Ba^T via fused ACT bias-add.
 - per batch: inputs[b] --gpsimd cast-DMA--> SBUF fp16 natural [T,D];
   -> DRAM fp16 scratch -> XBAR dma transpose -> X^T fp16 [D,T].
   S^T[mc] = sum_dc Ua^T[dc,mc] @ X^T[dc]  (fp16 matmuls, N=512, psum f32)
   tanh fused on ACT with per-partition bias (WaS+Ba)^T -> S^T fp16 in SBUF.
   z = sum_mc Va^T[mc] @ S^T[mc]  (fp16, M=1) -> relu on ACT -> zall[b].
 - softmax over T on zall [8,512] (DVE+ACT), cast sm to fp16.
 - sm^T via PE transpose; ctx[b] = sum_tc sm^T[tc,b] @ X_nat[tc] (fp16, M=1).
"""

import numpy as np

import concourse.bass as bass
import concourse.tile as tile
import concourse.mybir as mybir
from concourse import bacc
from concourse.bass import ts
from concourse.bass_utils import run_bass_kernel_spmd
from concourse.masks import make_identity

f32 = mybir.dt.float32
f16 = mybir.dt.float16

N_CORES = 8
B = 8          # batches per core
T = 512
D = 1024
OUT = 1024
P = 128
DC = D // P    # 8 contraction chunks
MC = OUT // P  # 8 out-tile chunks
TC = T // P    # 4 t chunks
NS = 512       # matmul free-dim slice


def build_bass():
    nc = bacc.Bacc("TRN2", target_bir_lowering=False, debug=False,
                   num_devices=N_CORES)

    inputs = nc.dram_tensor("inputs", [B, T, D], f32, kind="ExternalInput").ap()
    prev = nc.dram_tensor("prev_state", [B, OUT], f32, kind="ExternalInput").ap()
    Wa = nc.dram_tensor("Wa", [OUT, OUT], f32, kind="ExternalInput").ap()
    Ua = nc.dram_tensor("Ua", [D, OUT], f32, kind="ExternalInput").ap()
    Va = nc.dram_tensor("Va", [OUT, 1], f32, kind="ExternalInput").ap()
    Ba = nc.dram_tensor("Ba", [1, OUT], f32, kind="ExternalInput").ap()
    out = nc.dram_tensor("out", [B, D], f32, kind="ExternalOutput").ap()

    with tile.TileContext(nc) as tc:
        with (
            tc.tile_pool(name="const", bufs=1) as const,
            tc.tile_pool(name="work", bufs=2) as work,
            tc.tile_pool(name="nat", bufs=B) as natp,
            tc.tile_pool(name="ps_big", bufs=4, space="PSUM") as ps_big,
            tc.tile_pool(name="ps_small", bufs=3, space="PSUM") as ps_small,
            tc.tile_pool(name="dram", bufs=3, space="DRAM") as dram,
        ):
            # ---- small loads + weights first: HWDGE rings are in-order, so
            # nothing input-dependent may precede these on either ring ----
            prev_sb = const.tile([B, OUT], f32)
            nc.sync.dma_start(prev_sb[:], prev[:])
            BaT_sb = const.tile([P, MC], f32)
            nc.sync.dma_start(BaT_sb[:], Ba.rearrange("one (c p) -> p (one c)", p=P))
            Va_f32 = const.tile([P, MC], f32)
            nc.sync.dma_start(Va_f32[:], Va.rearrange("(c p) one -> p (c one)", p=P))
            Va_sb = const.tile([P, MC], f16)
            nc.vector.tensor_copy(Va_sb[:], Va_f32[:])
            prev16 = const.tile([B, OUT], f16)
            nc.vector.tensor_copy(prev16[:], prev_sb[:])

            # Ua fp32 halves on the two HWDGE rings, cast on DVE
            Ua_sb = const.tile([P, DC, OUT], f16)
            uh0 = work.tile([P, DC // 2, OUT], f32, tag="uastage", bufs=2)
            nc.sync.dma_start(
                uh0[:], Ua[:D // 2].rearrange("(c p) o -> p c o", p=P))
            uh1 = work.tile([P, DC // 2, OUT], f32, tag="uastage", bufs=2)
            nc.scalar.dma_start(
                uh1[:], Ua[D // 2:].rearrange("(c p) o -> p c o", p=P))
            nc.vector.tensor_copy(Ua_sb[:, :DC // 2, :], uh0[:])
            nc.vector.tensor_copy(Ua_sb[:, DC // 2:, :], uh1[:])

            ident = const.tile([P, P], f32)
            make_identity(nc, ident)
            ident16 = const.tile([P, P], f16)
            make_identity(nc, ident16)

            # ---- input chains ----
            # gpsimd: fp32->fp16 cast DMA into SBUF natural layout
            # scalar ring: SBUF -> DRAM bounce in SBUF-matching layout
            #              (per-partition 8KB contiguous runs)
            # sync ring: 4 XBAR transposes (one per 128-row t-chunk)
            nat16_tiles = []
            xt_tiles = {}

            def start_input_chain(b):
                nat16 = natp.tile([P, TC, D], f16, tag="nat16")
                nat16_tiles.append(nat16)
                nc.gpsimd.dma_start(
                    nat16[:], inputs[b].rearrange("(c p) d -> p c d", p=P))
                nat_dram = dram.tile([P, TC, D], f16, tag="natdram")
                nc.scalar.dma_start(nat_dram[:], nat16[:])
                xt = work.tile([P, DC, T], f16, tag="xt")
                xt_tiles[b] = xt
                for tcI in range(TC):
                    nc.sync.dma_start_transpose(
                        xt[:, :, ts(tcI, P)], nat_dram[:, tcI, :])

            start_input_chain(0)

            # Wa as fp16 via gpsimd cast-DMA, between input casts 0 and 1
            Wa_sb = const.tile([P, MC, OUT], f16)
            nc.gpsimd.dma_start(Wa_sb[:], Wa.rearrange("(c p) o -> p c o", p=P))

            start_input_chain(1)

            # prevT (fp16) via PE transposes
            prevT_sb = const.tile([P, MC, B], f16)
            for oc in range(MC):
                pt_ps = ps_small.tile([P, B], f16, tag="psm")
                nc.tensor.transpose(pt_ps[:], prev16[:, ts(oc, P)], ident16[:B, :B])
                nc.vector.tensor_copy(prevT_sb[:, oc, :], pt_ps[:])

            WaSBaT_sb = const.tile([P, MC, B], f32)
            smT_sb = const.tile([P, TC, B], f16)

            def emit_was_prep():
                # WaS natural [b, p] = prev @ Wa with prevT as the stationary
                # operand (8-col LDWEIGHTS, wide fp16 matmuls)
                wasnat_sb = const.tile([B, OUT], f32)
                for n in range(OUT // NS):
                    was_ps = ps_small.tile([B, NS], f32, tag="psm")
                    for oc in range(MC):
                        nc.tensor.matmul(was_ps[:], prevT_sb[:, oc, :],
                                         Wa_sb[:, oc, ts(n, NS)],
                                         start=(oc == 0), stop=(oc == MC - 1))
                    nc.vector.tensor_copy(wasnat_sb[:, ts(n, NS)], was_ps[:])
                # transpose to [p, mc, b] and add Ba^T on ACT
                for mc in range(MC):
                    wt_ps = ps_small.tile([P, B], f32, tag="psm")
                    nc.tensor.transpose(wt_ps[:], wasnat_sb[:, ts(mc, P)],
                                        ident[:B, :B])
                    nc.scalar.activation(WaSBaT_sb[:, mc, :], wt_ps[:],
                                         mybir.ActivationFunctionType.Identity,
                                         bias=BaT_sb[:, mc:mc + 1], scale=1.0)

            # ---------------- fully pipelined per-batch flow ----------------
            for b in range(B):
                if b + 2 < B:
                    start_input_chain(b + 2)
                xt = xt_tiles[b]

                st = work.tile([P, MC, T], f16, tag="st")
                deferred = []
                for mc in range(MC):
                    st_ps = ps_big.tile([P, NS], f32, tag="stps")
                    for dc in range(DC):
                        nc.tensor.matmul(st_ps[:], Ua_sb[:, dc, ts(mc, P)],
                                         xt[:, dc, :],
                                         start=(dc == 0), stop=(dc == DC - 1))
                    if b == 0 and mc <= 2:
                        # batch 0's first tanhs must be emitted after the WaS
                        # prep writes WaSBaT (reads before writes in trace
                        # order get no RAW edge)
                        deferred.append((mc, st_ps))
                        if mc == 2:
                            # Wa cast-DMA lands ~20us in; slot the WaS prep
                            # into the PE stream here so nothing stalls
                            emit_was_prep()
                            for mcd, psd in deferred:
                                nc.scalar.activation(
                                    st[:, mcd, :], psd[:],
                                    mybir.ActivationFunctionType.Tanh,
                                    bias=WaSBaT_sb[:, mcd, b:b + 1], scale=1.0)
                    else:
                        nc.scalar.activation(st[:, mc, :], st_ps[:],
                                             mybir.ActivationFunctionType.Tanh,
                                             bias=WaSBaT_sb[:, mc, b:b + 1],
                                             scale=1.0)

                z_ps = ps_small.tile([1, T], f32, tag="psm")
                for mc in range(MC):
                    nc.tensor.matmul(z_ps[:], Va_sb[:, mc:mc + 1], st[:, mc, :],
                                     start=(mc == 0), stop=(mc == MC - 1))
                z_sb = work.tile([1, T], f32, tag="zsb")
                nc.scalar.activation(z_sb[:], z_ps[:],
                                     mybir.ActivationFunctionType.Relu)

                # per-batch softmax over T (1 partition, small)
                negmax = work.tile([1, 1], f32, tag="nm")
                nc.vector.reduce_max(negmax[:], z_sb[:],
                                     axis=mybir.AxisListType.X, negate=True)
                esb = work.tile([1, T], f32, tag="esb")
                nc.scalar.activation(esb[:], z_sb[:],
                                     mybir.ActivationFunctionType.Exp,
                                     bias=negmax[:], scale=1.0)
                ssum = work.tile([1, 1], f32, tag="ss")
                nc.vector.reduce_sum(ssum[:], esb[:], axis=mybir.AxisListType.X)
                rsum = work.tile([1, 1], f32, tag="rs")
                nc.vector.reciprocal(rsum[:], ssum[:])
                sm16 = work.tile([1, T], f16, tag="sm16")
                nc.vector.tensor_scalar_mul(sm16[:], esb[:], rsum[:])

                # sm^T for this batch: 4 PE transposes into one psum tile
                smt_ps = ps_small.tile([P, TC, 2], f16, tag="psm")
                for tcI in range(TC):
                    nc.tensor.transpose(smt_ps[:, tcI, 0:1],
                                        sm16[:, ts(tcI, P)], ident16[:1, :1])
                nc.vector.tensor_copy(smT_sb[:, :, b], smt_ps[:, :, 0])

                # ctx matmuls for this batch
                nat16 = nat16_tiles[b]
                ctx_sb = work.tile([1, D], f32, tag="ctx")
                for n in range(D // NS):
                    ctx_ps = ps_small.tile([1, NS], f32, tag="psm")
                    for tcI in range(TC):
                        nc.tensor.matmul(ctx_ps[:], smT_sb[:, tcI, b:b + 1],
                                         nat16[:, tcI, ts(n, NS)],
                                         start=(tcI == 0), stop=(tcI == TC - 1))
                    nc.vector.tensor_copy(ctx_sb[:, ts(n, NS)], ctx_ps[:])
                nc.sync.dma_start(out[b:b + 1, :], ctx_sb[:])

    nc.compile()
    return nc


_NC = None


def _get_nc():
    global _NC
    if _NC is None:
        _NC = build_bass()
    return _NC


def run(inputs, prev_state, Wa, Ua, Va, Ba, **spmd_kwargs):
    nc = _get_nc()
    inputs = np.ascontiguousarray(inputs, dtype=np.float32)
    prev_state = np.ascontiguousarray(prev_state, dtype=np.float32)
    weights = {
        "Wa": np.ascontiguousarray(Wa, dtype=np.float32),
        "Ua": np.ascontiguousarray(Ua, dtype=np.float32),
        "Va": np.ascontiguousarray(Va, dtype=np.float32),
        "Ba": np.ascontiguousarray(Ba, dtype=np.float32),
    }
    in_maps = []
    for c in range(N_CORES):
        sl = slice(c * B, (c + 1) * B)
        in_maps.append({
            "inputs": inputs[sl],
            "prev_state": prev_state[sl],
            **weights,
        })
    return run_bass_kernel_spmd(nc, in_maps, core_ids=list(range(N_CORES)),
                                **spmd_kwargs)


def kernel(inputs, prev_state, Wa, Ua, Va, Ba):
    res = run(inputs, prev_state, Wa, Ua, Va, Ba)
    return np.concatenate([r["out"] for r in res.results], axis=0)


# revision 14
# speedup vs baseline: 1.0567x; 1.0567x over previous
"""CascadedAttentionCell Trainium2 kernel.

Full shapes: inputs [64, 512, 1024] f32, prev_state [64, 1024] f32,
Wa [1024,1024], Ua [1024,1024], Va [1024,1], Ba [1,1024].
Output: context vector [64, 1024] f32.

Sharding: data-parallel over batch across 8 NeuronCores (8 batches/core);
weights replicated.

Per-core plan (B=8 local batches, T=512, D=1024, OUT=1024, P=128):
 - prep: Ua -> SBUF fp16; Wa -> SBUF f32; prev_state transposed via PE;
   WaS^T = Wa^T @ prev^T (fp32 matmuls, N=8); + Ba^T via fused ACT bias-add.
 - per batch: inputs[b] --gpsimd cast-DMA--> SBUF fp16 natural [T,D];
   -> DRAM fp16 scratch -> XBAR dma transpose -> X^T fp16 [D,T].
   S^T[mc] = sum_dc Ua^T[dc,mc] @ X^T[dc]  (fp16 matmuls, N=512, psum f32)
   tanh fused on ACT with per-partition bias (WaS+Ba)^T -> S^T fp16 in SBUF.
   z = sum_mc Va^T[mc] @ S^T[mc]  (fp16, M=1) -> relu on ACT -> zall[b].
 - softmax over T on zall [8,512] (DVE+ACT), cast sm to fp16.
 - sm^T via PE transpose; ctx[b] = sum_tc sm^T[tc,b] @ X_nat[tc] (fp16, M=1).
"""

import numpy as np

import concourse.bass as bass
import concourse.tile as tile
import concourse.mybir as mybir
from concourse import bacc
from concourse.bass import ts
from concourse.bass_utils import run_bass_kernel_spmd
from concourse.masks import make_identity

f32 = mybir.dt.float32
f16 = mybir.dt.float16

N_CORES = 8
B = 8          # batches per core
T = 512
D = 1024
OUT = 1024
P = 128
DC = D // P    # 8 contraction chunks
MC = OUT // P  # 8 out-tile chunks
TC = T // P    # 4 t chunks
NS = 512       # matmul free-dim slice


def build_bass():
    nc = bacc.Bacc("TRN2", target_bir_lowering=False, debug=False,
                   num_devices=N_CORES)

    inputs = nc.dram_tensor("inputs", [B, T, D], f32, kind="ExternalInput").ap()
    prev = nc.dram_tensor("prev_state", [B, OUT], f32, kind="ExternalInput").ap()
    Wa = nc.dram_tensor("Wa", [OUT, OUT], f32, kind="ExternalInput").ap()
    Ua = nc.dram_tensor("Ua", [D, OUT], f32, kind="ExternalInput").ap()
    Va = nc.dram_tensor("Va", [OUT, 1], f32, kind="ExternalInput").ap()
    Ba = nc.dram_tensor("Ba", [1, OUT], f32, kind="ExternalInput").ap()
    out = nc.dram_tensor("out", [B, D], f32, kind="ExternalOutput").ap()

    with tile.TileContext(nc) as tc:
        with (
            tc.tile_pool(name="const", bufs=1) as const,
            tc.tile_pool(name="work", bufs=2) as work,
            tc.tile_pool(name="nat", bufs=B) as natp,
            tc.tile_pool(name="ps_big", bufs=4, space="PSUM") as ps_big,
            tc.tile_pool(name="ps_xt", bufs=2, space="PSUM") as ps_xt,
            tc.tile_pool(name="ps_small", bufs=2, space="PSUM") as ps_small,
        ):
            # ---- small loads first (HWDGE rings are in-order) ----
            prev_sb = const.tile([B, OUT], f32)
            nc.sync.dma_start(prev_sb[:], prev[:])
            BaT_sb = const.tile([P, MC], f32)
            nc.sync.dma_start(BaT_sb[:], Ba.rearrange("one (c p) -> p (one c)", p=P))
            Va_f32 = const.tile([P, MC], f32)
            nc.sync.dma_start(Va_f32[:], Va.rearrange("(c p) one -> p (c one)", p=P))
            Va_sb = const.tile([P, MC], f16)
            nc.vector.tensor_copy(Va_sb[:], Va_f32[:])
            prev16 = const.tile([B, OUT], f16)
            nc.vector.tensor_copy(prev16[:], prev_sb[:])

            # Ua fp32 halves on the two HWDGE rings, cast on DVE
            Ua_sb = const.tile([P, DC, OUT], f16)
            uh0 = work.tile([P, DC // 2, OUT], f32, tag="stage", bufs=2)
            nc.sync.dma_start(
                uh0[:], Ua[:D // 2].rearrange("(c p) o -> p c o", p=P))
            uh1 = work.tile([P, DC // 2, OUT], f32, tag="stage", bufs=2)
            nc.scalar.dma_start(
                uh1[:], Ua[D // 2:].rearrange("(c p) o -> p c o", p=P))
            nc.vector.tensor_copy(Ua_sb[:, :DC // 2, :], uh0[:])
            nc.vector.tensor_copy(Ua_sb[:, DC // 2:, :], uh1[:])

            ident = const.tile([P, P], f32)
            make_identity(nc, ident)
            ident16 = const.tile([P, P], f16)
            make_identity(nc, ident16)

            # ---- input loads ----
            # even batches: gpsimd cast-DMA straight to fp16 SBUF
            # odd batches: fp32 via sync ring + DVE cast (keeps the SWDGE
            # ring free for Wa)
            nat16_tiles = {}

            def load_input(b):
                nat16 = natp.tile([P, TC, D], f16, tag="nat16")
                nat16_tiles[b] = nat16
                if b % 2 == 0:
                    nc.gpsimd.dma_start(
                        nat16[:], inputs[b].rearrange("(c p) d -> p c d", p=P))
                else:
                    stg = work.tile([P, TC, D], f32, tag="stage", bufs=2)
                    nc.sync.dma_start(
                        stg[:], inputs[b].rearrange("(c p) d -> p c d", p=P))
                    nc.vector.tensor_copy(nat16[:], stg[:])

            load_input(0)
            load_input(1)

            # Wa fp16 via two gpsimd cast-DMAs (after input cast 0)
            Wa_sb = const.tile([P, MC, OUT], f16)
            nc.gpsimd.dma_start(
                Wa_sb[:, :MC // 2, :],
                Wa[:OUT // 2].rearrange("(c p) o -> p c o", p=P))
            nc.gpsimd.dma_start(
                Wa_sb[:, MC // 2:, :],
                Wa[OUT // 2:].rearrange("(c p) o -> p c o", p=P))

            load_input(2)
            load_input(3)

            # prevT (fp16) via PE transposes
            prevT_sb = const.tile([P, MC, B], f16)
            for oc in range(MC):
                pt_ps = ps_small.tile([P, B], f16, tag="psm")
                nc.tensor.transpose(pt_ps[:], prev16[:, ts(oc, P)], ident16[:B, :B])
                nc.vector.tensor_copy(prevT_sb[:, oc, :], pt_ps[:])

            WaSBaT_sb = const.tile([P, MC, B], f32)
            smT_sb = const.tile([P, TC, B], f16)
            xt_tiles = {}

            def emit_xpose(b):
                # X^T built on PE: 32 [128,128] fp16 transposes, copied out
                # of PSUM by DVE in [128,512] chunks
                nat16 = nat16_tiles[b]
                xt = work.tile([P, DC, T], f16, tag="xt")
                xt_tiles[b] = xt
                for dc in range(DC):
                    xt_ps = ps_xt.tile([P, T], f16, tag="xtps")
                    for tcI in range(TC):
                        nc.tensor.transpose(xt_ps[:, ts(tcI, P)],
                                            nat16[:, tcI, ts(dc, P)],
                                            ident16[:])
                    nc.vector.tensor_copy(xt[:, dc, :], xt_ps[:])

            def emit_was_prep():
                # WaS natural [b, p] = prev @ Wa with prevT stationary
                wasnat_sb = const.tile([B, OUT], f32)
                for n in range(OUT // NS):
                    was_ps = ps_small.tile([B, NS], f32, tag="psm")
                    for oc in range(MC):
                        nc.tensor.matmul(was_ps[:], prevT_sb[:, oc, :],
                                         Wa_sb[:, oc, ts(n, NS)],
                                         start=(oc == 0), stop=(oc == MC - 1))
                    nc.vector.tensor_copy(wasnat_sb[:, ts(n, NS)], was_ps[:])
                for mc in range(MC):
                    wt_ps = ps_small.tile([P, B], f32, tag="psm")
                    nc.tensor.transpose(wt_ps[:], wasnat_sb[:, ts(mc, P)],
                                        ident[:B, :B])
                    nc.scalar.activation(WaSBaT_sb[:, mc, :], wt_ps[:],
                                         mybir.ActivationFunctionType.Identity,
                                         bias=BaT_sb[:, mc:mc + 1], scale=1.0)

            emit_xpose(0)
            emit_xpose(1)

            # ---------------- fully pipelined per-batch flow ----------------
            for b in range(B):
                if b + 4 < B:
                    load_input(b + 4)
                xt = xt_tiles[b]

                st = work.tile([P, MC, T], f16, tag="st")
                deferred = []
                for mc in range(MC):
                    st_ps = ps_big.tile([P, NS], f32, tag="stps")
                    for dc in range(DC):
                        nc.tensor.matmul(st_ps[:], Ua_sb[:, dc, ts(mc, P)],
                                         xt[:, dc, :],
                                         start=(dc == 0), stop=(dc == DC - 1))
                    if b == 0:
                        # batch 0's tanhs wait for the WaS prep (Wa cast-DMA
                        # lands ~35us in); defer them so reads of WaSBaT are
                        # emitted after its writes
                        deferred.append((mc, st_ps))
                    else:
                        nc.scalar.activation(st[:, mc, :], st_ps[:],
                                             mybir.ActivationFunctionType.Tanh,
                                             bias=WaSBaT_sb[:, mc, b:b + 1],
                                             scale=1.0)
                if b == 0:
                    emit_was_prep()
                    for mcd, psd in deferred:
                        nc.scalar.activation(
                            st[:, mcd, :], psd[:],
                            mybir.ActivationFunctionType.Tanh,
                            bias=WaSBaT_sb[:, mcd, b:b + 1], scale=1.0)

                # transpose for the next batch rides after this batch's MMs
                if b + 1 < B:
                    emit_xpose(b + 1)

                z_ps = ps_small.tile([1, T], f32, tag="psm")
                for mc in range(MC):
                    nc.tensor.matmul(z_ps[:], Va_sb[:, mc:mc + 1], st[:, mc, :],
                                     start=(mc == 0), stop=(mc == MC - 1))
                z_sb = work.tile([1, T], f32, tag="zsb")
                nc.scalar.activation(z_sb[:], z_ps[:],
                                     mybir.ActivationFunctionType.Relu)

                # per-batch softmax over T (1 partition, small)
                negmax = work.tile([1, 1], f32, tag="nm")
                nc.vector.reduce_max(negmax[:], z_sb[:],
                                     axis=mybir.AxisListType.X, negate=True)
                esb = work.tile([1, T], f32, tag="esb")
                nc.scalar.activation(esb[:], z_sb[:],
                                     mybir.ActivationFunctionType.Exp,
                                     bias=negmax[:], scale=1.0)
                ssum = work.tile([1, 1], f32, tag="ss")
                nc.vector.reduce_sum(ssum[:], esb[:], axis=mybir.AxisListType.X)
                rsum = work.tile([1, 1], f32, tag="rs")
                nc.vector.reciprocal(rsum[:], ssum[:])
                sm16 = work.tile([1, T], f16, tag="sm16")
                nc.vector.tensor_scalar_mul(sm16[:], esb[:], rsum[:])

                # sm^T for this batch: 4 PE transposes into one psum tile
                smt_ps = ps_small.tile([P, TC, 2], f16, tag="psm")
                for tcI in range(TC):
                    nc.tensor.transpose(smt_ps[:, tcI, 0:1],
                                        sm16[:, ts(tcI, P)], ident16[:1, :1])
                nc.vector.tensor_copy(smT_sb[:, :, b], smt_ps[:, :, 0])

                # ctx matmuls for this batch
                nat16 = nat16_tiles[b]
                ctx_sb = work.tile([1, D], f32, tag="ctx")
                for n in range(D // NS):
                    ctx_ps = ps_small.tile([1, NS], f32, tag="psm")
                    for tcI in range(TC):
                        nc.tensor.matmul(ctx_ps[:], smT_sb[:, tcI, b:b + 1],
                                         nat16[:, tcI, ts(n, NS)],
                                         start=(tcI == 0), stop=(tcI == TC - 1))
                    nc.vector.tensor_copy(ctx_sb[:, ts(n, NS)], ctx_ps[:])
                nc.sync.dma_start(out[b:b + 1, :], ctx_sb[:])

    nc.compile()
    return nc


_NC = None


def _get_nc():
    global _NC
    if _NC is None:
        _NC = build_bass()
    return _NC


def run(inputs, prev_state, Wa, Ua, Va, Ba, **spmd_kwargs):
    nc = _get_nc()
    inputs = np.ascontiguousarray(inputs, dtype=np.float32)
    prev_state = np.ascontiguousarray(prev_state, dtype=np.float32)
    weights = {
        "Wa": np.ascontiguousarray(Wa, dtype=np.float32),
        "Ua": np.ascontiguousarray(Ua, dtype=np.float32),
        "Va": np.ascontiguousarray(Va, dtype=np.float32),
        "Ba": np.ascontiguousarray(Ba, dtype=np.float32),
    }
    in_maps = []
    for c in range(N_CORES):
        sl = slice(c * B, (c + 1) * B)
        in_maps.append({
            "inputs": inputs[sl],
            "prev_state": prev_state[sl],
            **weights,
        })
    return run_bass_kernel_spmd(nc, in_maps, core_ids=list(range(N_CORES)),
                                **spmd_kwargs)


def kernel(inputs, prev_state, Wa, Ua, Va, Ba):
    res = run(inputs, prev_state, Wa, Ua, Va, Ba)
    return np.concatenate([r["out"] for r in res.results], axis=0)
